# revision 1
# baseline (speedup 1.0000x reference)
"""Trainium2 Bass kernel for nn_Attention_54520314855575.

GQA attention with raw row-major reshapes (faithful to reference). The raw
reshapes scramble heads/tokens such that each query head's 64 output rows are
disjoint across heads -> shard 8 ways (2 batches x 4 head-groups) with zero
collectives. Per core: 8 query heads (hq%8 in {2r, 2r+1}), K/V heads {2r,2r+1}.

Compute: bf16 matmuls, f32 PSUM/softmax. All projections computed transposed
(channels on partitions) so biases are per-partition; V computed natural and
shuffled on-chip into (t, d) layout.

t-axis permutation: within each 128-row t-tile, partition p holds t-offset
8*(p%16) + p//16 (so the V shuffle writes 16 contiguous partitions per
c-chunk). The scores lhsT (KT) is stored in the same order and the diagonal
mask rows are permuted on host, so the contraction stays consistent.

Weights are host-pretiled so every weight-slab DMA is one contiguous block.
"""
import sys, os

for _p in ("/opt/trn_rl_repo",):
    if _p not in sys.path:
        sys.path.append(_p)

import numpy as np
import ml_dtypes

import concourse.bass as bass
import concourse.tile as tile
from concourse import bacc, mybir
from concourse.bass_utils import run_bass_kernel_spmd

BF16 = mybir.dt.bfloat16
F32 = mybir.dt.float32

H = 4096; HQ = 32; HK = 8; HV = 8; DQ = 128; DV = 512; S = 2048; B = 2
NEG = -1.0e30

_CACHE = {}


def build(causal: bool):
    nc = bacc.Bacc(None, target_bir_lowering=False, debug=False)

    xq_d = nc.declare_dram_parameter("xq", [128, 32, 512], BF16, isOutput=False)
    xkv_d = nc.declare_dram_parameter("xkv", [128, 32, 512], BF16, isOutput=False)
    wq_d = nc.declare_dram_parameter("wq", [8, 32, 128, 512], BF16, isOutput=False)
    bq_d = nc.declare_dram_parameter("bq", [128, 32], F32, isOutput=False)
    wk_d = nc.declare_dram_parameter("wk", [2, 32, 128, 512], BF16, isOutput=False)
    bk_d = nc.declare_dram_parameter("bk", [128, 8], F32, isOutput=False)
    wv_d = nc.declare_dram_parameter("wv", [8, 32, 128, 512], BF16, isOutput=False)
    bV_d = nc.declare_dram_parameter("bV", [128, 512], BF16, isOutput=False)
    w0_d = nc.declare_dram_parameter("w0", [4, 128, 128, 1024], BF16, isOutput=False)
    bias_plane_d = nc.declare_dram_parameter("bias_plane", [4096, 512], F32, isOutput=False)
    mask_diag_d = nc.declare_dram_parameter("mask_diag", [128, 128], F32, isOutput=False)
    if not causal:
        maskT_d = nc.declare_dram_parameter("maskT", [16, 128, 4, 512], BF16, isOutput=False)
    outT_d = nc.declare_dram_parameter("outT", [4096, 512], F32, isOutput=True)

    with tile.TileContext(nc) as tc:
        with tc.tile_pool(name="const", bufs=1) as constp, \
             tc.tile_pool(name="qkv", bufs=1) as qkvp, \
             tc.tile_pool(name="dram", bufs=1, space="DRAM") as dramp:

            mask_sb = constp.tile([128, 128], F32)
            nc.sync.dma_start(mask_sb[:], mask_diag_d[:])
            bV_sb = constp.tile([128, 512], BF16)
            nc.sync.dma_start(bV_sb[:], bV_d[:])
            bq_sb = constp.tile([128, 32], F32)
            nc.sync.dma_start(bq_sb[:], bq_d[:])
            bk_sb = constp.tile([128, 8], F32)
            nc.sync.dma_start(bk_sb[:], bk_d[:])
            ones_f = constp.tile([128, 1], F32)
            nc.vector.memset(ones_f[:], 1.0)
            ones_r = constp.tile([128, 1], mybir.dt.float32r)
            nc.vector.tensor_copy(ones_r[:], ones_f[:])

            QT = qkvp.tile([128, 8, 2048], BF16)   # [d, head hloc, q]
            KT = qkvp.tile([128, 2, 2048], BF16)   # [d, head j0, tperm]
            Vsh = qkvp.tile([128, 2, 16, 512], BF16)  # [pnew, head j0, ttile, d]
            ctx_dram = dramp.tile([32, 4, 128, 512], BF16)  # [sm, dd, dpart, s']
            vf_dram = dramp.tile([512, 4096], BF16)         # natural (tokloc, c)

            # ---------------- Phase 1: projections ----------------
            with tc.tile_pool(name="xres", bufs=1) as xp, \
                 tc.tile_pool(name="wstr", bufs=4) as wp, \
                 tc.tile_pool(name="vtmp", bufs=3) as vtp, \
                 tc.tile_pool(name="pps", bufs=8, space="PSUM") as pps:

                xq_sb = xp.tile([128, 32, 512], BF16)
                nc.sync.dma_start(xq_sb[:], xq_d[:])
                xkv_sb = xp.tile([128, 32, 512], BF16)
                nc.sync.dma_start(xkv_sb[:], xkv_d[:])

                # QT: lhsT = wq block (h128, c128), rhs = xq (h128, tok512)
                for cg in range(8):
                    acc = [pps.tile([128, 512], F32, tag="pj", name=f"pj{_}") for _ in range(4)]
                    for h in range(32):
                        wsl = wp.tile([128, 512], BF16, tag="w")
                        nc.sync.dma_start(wsl[:], wq_d[cg, h, :, :])
                        for i in range(4):
                            nc.tensor.matmul(acc[i][:], wsl[:, 128*i:128*i+128],
                                             xq_sb[:, h, :], start=(h == 0), stop=(h == 31))
                    for i in range(4):
                        ct = 4*cg + i  # == sm
                        # QT[p, hd, q=32u+sm] <- acc[p, tok=128k+64j0+u], hd=2k+j0
                        out = QT[:].rearrange("p hd (u sm) -> p hd u sm", sm=32)[:, :, :, ct]
                        nc.vector.tensor_scalar_add(
                            out, acc[i][:].rearrange("p (hd u) -> p hd u", hd=8),
                            bq_sb[:, ct:ct+1])

                # KT: new within-tile t order: free = 128*i4 + 16*cc + m
                for cg in range(2):
                    acc = [pps.tile([128, 512], F32, tag="pj", name=f"pj{_}") for _ in range(4)]
                    for h in range(32):
                        wsl = wp.tile([128, 512], BF16, tag="w")
                        nc.sync.dma_start(wsl[:], wk_d[cg, h, :, :])
                        for i in range(4):
                            nc.tensor.matmul(acc[i][:], wsl[:, 128*i:128*i+128],
                                             xkv_sb[:, h, :], start=(h == 0), stop=(h == 31))
                    for i in range(4):
                        ct = 4*cg + i  # == cc
                        # KT[p, hd, 128*i4 + 16*cc + m] <- acc[p, tok=256*hd+16*i4+m]
                        out = KT[:].rearrange("p hd (i4 cc m) -> p hd i4 cc m",
                                              cc=8, m=16)[:, :, :, ct, :]
                        nc.vector.tensor_scalar_add(
                            out, acc[i][:].rearrange("p (hd i4 m) -> p hd i4 m", hd=2, i4=16),
                            bk_sb[:, ct:ct+1])

                # V natural: lhsT = xkv block (h128, tok128), rhs = wv (h128, c512)
                for ccg in range(8):
                    acc = [pps.tile([128, 512], F32, tag="pj", name=f"pj{_}") for _ in range(4)]
                    for h in range(32):
                        wsl = wp.tile([128, 512], BF16, tag="w")
                        nc.sync.dma_start(wsl[:], wv_d[ccg, h, :, :])
                        for tt in range(4):
                            nc.tensor.matmul(acc[tt][:], xkv_sb[:, h, 128*tt:128*tt+128],
                                             wsl[:], start=(h == 0), stop=(h == 31))
                    for tt in range(4):
                        vnat = vtp.tile([128, 512], BF16, tag="vn")
                        nc.vector.tensor_copy(vnat[:], acc[tt][:])
                        nc.sync.dma_start(
                            vf_dram[128*tt:128*tt+128, 512*ccg:512*ccg+512], vnat[:])

                # gather V (t, d) tiles from DRAM: pnew = 16*cc + m holds
                # t = 128*i4 + 8*m + cc, i.e. Vf[256*j0 + 16*i4 + m, cc*512 + d]
                for j0 in range(2):
                    for i4 in range(16):
                        for cc in range(8):
                            r0_ = 256*j0 + 16*i4
                            nc.sync.dma_start(
                                Vsh[16*cc:16*cc+16, j0, i4, :],
                                vf_dram[r0_:r0_+16, 512*cc:512*cc+512])

                # V bias: V[pnew, d] += bV[pnew, d] (host permuted)
                for j0 in range(2):
                    for i4 in range(16):
                        nc.vector.tensor_add(Vsh[:, j0, i4, :], Vsh[:, j0, i4, :], bV_sb[:])

            # ---------------- Phase 2: attention ----------------
            with tc.tile_pool(name="esb", bufs=2) as ep, \
                 tc.tile_pool(name="nrm", bufs=2) as np_, \
                 tc.tile_pool(name="cev", bufs=4) as cevp, \
                 tc.tile_pool(name="mstr", bufs=4) as mp, \
                 tc.tile_pool(name="aps", bufs=1, space="PSUM") as aps:

                for hloc in range(8):
                    k, j0 = hloc // 2, hloc % 2
                    for c in range(4):
                        nt = 4*c + 4 if causal else 16
                        E = ep.tile([128, 16, 512], BF16, tag="E")
                        Esum = ep.tile([128, 512], mybir.dt.float32r, tag="Esum")
                        rs_ps = aps.tile([1, 512], F32, tag="rs")
                        pctx = [aps.tile([128, 512], F32, tag=f"ctx{dd}", name=f"ctx{dd}")
                                for dd in range(4)]
                        for i4 in range(nt):
                            sc_ps = aps.tile([128, 512], F32, tag="sc", bufs=2)
                            nc.tensor.matmul(
                                sc_ps[:], KT[:, j0, 128*i4:128*i4+128],
                                QT[:, hloc, 512*c:512*c+512], start=True, stop=True)
                            if causal:
                                if 4*c <= i4 < 4*c+4:
                                    q0 = 128*(i4 - 4*c)
                                    nc.vector.tensor_add(sc_ps[:, q0:q0+128],
                                                         sc_ps[:, q0:q0+128], mask_sb[:])
                                m0 = 128*(i4 - 4*c) if i4 > 4*c else 0
                            else:
                                msk = mp.tile([128, 512], BF16, tag="mk")
                                nc.sync.dma_start(msk[:], maskT_d[i4, :, c, :])
                                nc.vector.tensor_add(sc_ps[:], sc_ps[:], msk[:])
                                m0 = 0
                            if m0 > 0:
                                nc.vector.memset(E[:, i4, 0:m0], 0.0)
                            nc.scalar.activation(E[:, i4, m0:512], sc_ps[:, m0:512],
                                                 mybir.ActivationFunctionType.Exp)
                            if i4 == 0:
                                nc.vector.tensor_copy(Esum[:], E[:, 0, :])
                            else:
                                nc.vector.tensor_add(Esum[:], Esum[:], E[:, i4, :])
                            for dd in range(4):
                                nc.tensor.matmul(pctx[dd][:],
                                                 Vsh[:, j0, i4, 128*dd:128*dd+128],
                                                 E[:, i4, :],
                                                 start=(i4 == 0), stop=(i4 == nt-1))
                        nc.tensor.matmul(rs_ps[:], ones_r[:], Esum[:],
                                         start=True, stop=True)
                        # short normalize chain: recip on (1,512), then broadcast
                        rs_sb = np_.tile([1, 512], F32, tag="rssb")
                        nc.vector.tensor_copy(rs_sb[:], rs_ps[:])
                        rc1_sb = np_.tile([1, 512], F32, tag="rc1")
                        nc.vector.reciprocal(rc1_sb[:], rs_sb[:])
                        rc_sb = np_.tile([128, 512], F32, tag="rc")
                        nc.gpsimd.partition_broadcast(rc_sb[:], rc1_sb[:])
                        for dd in range(4):
                            # evict in sm-major order so the DRAM store is u-contiguous
                            cev = cevp.tile([128, 512], BF16, tag="cev")
                            perm = "p (u sm) -> p sm u"
                            nc.vector.tensor_mul(
                                cev[:],
                                pctx[dd][:].rearrange(perm, sm=32),
                                rc_sb[:].rearrange(perm, sm=32))
                            src = cev[:].rearrange("p (sm u) -> p sm u", u=16)
                            dst = ctx_dram[:].rearrange("sm dd dp s -> dp sm dd s")[
                                :, :, dd, 64*hloc+16*c:64*hloc+16*c+16]
                            nc.sync.dma_start(dst.opt(), src.opt())

            # ---------------- Phase 3: output projection ----------------
            with tc.tile_pool(name="w0str", bufs=4) as w0p, \
                 tc.tile_pool(name="rhsp", bufs=4) as rhp, \
                 tc.tile_pool(name="evo", bufs=3) as evp, \
                 tc.tile_pool(name="wps", bufs=1, space="PSUM") as wps:

                for og in range(4):
                    pout = [wps.tile([128, 512], F32, tag=f"o{o}", name=f"po{o}")
                            for o in range(8)]
                    for ft in range(128):
                        sm, dd = ft // 4, ft % 4
                        rhs = rhp.tile([128, 512], BF16, tag="rhs")
                        nc.sync.dma_start(rhs[:], ctx_dram[sm, dd, :, :])
                        wsl = w0p.tile([128, 1024], BF16, tag="w0")
                        nc.sync.dma_start(wsl[:], w0_d[og, ft, :, :])
                        for o in range(8):
                            nc.tensor.matmul(pout[o][:], wsl[:, 128*o:128*o+128], rhs[:],
                                             start=(ft == 0), stop=(ft == 127))
                    for o in range(8):
                        orow = 1024*og + 128*o
                        bsl = evp.tile([128, 512], F32, tag="bp")
                        nc.sync.dma_start(bsl[:], bias_plane_d[orow:orow+128, :])
                        res = evp.tile([128, 512], F32, tag="res")
                        nc.vector.tensor_add(res[:], pout[o][:], bsl[:])
                        nc.sync.dma_start(outT_d[orow:orow+128, :], res[:])

    nc.compile()
    return nc


def _tile_w(wT, ncg):
    """(4096h, ncg*512c) -> (ncg, 32, 128, 512) contiguous slabs."""
    hdim = wT.shape[0]
    return np.ascontiguousarray(
        wT.reshape(hdim // 128, 128, ncg, 512).transpose(2, 0, 1, 3))


def _prep(inputs):
    x = np.asarray(inputs["x"], np.float32)
    mask = np.asarray(inputs["mask"]).astype(bool)
    WQ_w = np.asarray(inputs["WQ_w"], np.float32); WQ_b = np.asarray(inputs["WQ_b"], np.float32)
    WK_w = np.asarray(inputs["WK_w"], np.float32); WK_b = np.asarray(inputs["WK_b"], np.float32)
    WV_w = np.asarray(inputs["WV_w"], np.float32); WV_b = np.asarray(inputs["WV_b"], np.float32)
    W0_w = np.asarray(inputs["W0_w"], np.float32); W0_b = np.asarray(inputs["W0_b"], np.float32)

    causal = bool(np.array_equal(mask, np.triu(np.ones((S, S), bool), k=1)))

    bf = ml_dtypes.bfloat16
    sc = 1.0 / np.sqrt(DQ)
    wq = _tile_w(np.ascontiguousarray((WQ_w * sc).T).astype(bf), 8)
    wk = _tile_w(np.ascontiguousarray(WK_w.T).astype(bf), 2)
    wv = _tile_w(np.ascontiguousarray(WV_w.T).astype(bf), 8)
    w0T = np.ascontiguousarray(W0_w.T).astype(bf)           # (16384, 4096)
    w0 = np.ascontiguousarray(
        w0T.reshape(128, 128, 4, 1024).transpose(2, 0, 1, 3))  # (og, ft, p, 1024)

    # t-permutation within a 128-tile: partition p holds t-offset 8*(p%16) + p//16
    pnew = np.arange(128)
    t_of_p = 8*(pnew % 16) + pnew // 16                     # (128,)

    # V bias (indexed by pnew): V[t, d] bias = WV_b[(t%8)*512 + d]; t%8 = t_of_p%8
    dd_ = np.arange(512)
    bV = WV_b[(t_of_p[:, None] % 8)*512 + dd_[None, :]].astype(bf)

    # diag mask rows permuted: masked iff t_of_p > qq
    qq_ = np.arange(128)
    mask_diag = np.where(t_of_p[:, None] > qq_[None, :], NEG, 0.0).astype(np.float32)

    plane = np.tile(W0_b[:, None], (1, 512)).astype(np.float32)

    maskT_perm = None
    if not causal:
        # maskT[i4, p, c, q'] additive, t = 128*i4 + t_of_p[p], q = 512*c + q'
        madd = np.where(mask.T, NEG, 0.0).astype(np.float32)  # (t, q)
        m4 = madd.reshape(16, 128, 4, 512)
        maskT_perm = np.ascontiguousarray(m4[:, t_of_p, :, :]).astype(bf)

    def fold(v, ntile):
        return np.ascontiguousarray(v.reshape(ntile, 128).T).astype(np.float32)

    bq = fold(WQ_b * sc, 32)
    bk = fold(WK_b, 8)

    in_maps = []
    meta = []
    for b in range(B):
        for r in range(4):
            qtok = np.concatenate(
                [np.arange(512*kk + 128*r, 512*kk + 128*r + 128) for kk in range(4)])
            kvtok = np.arange(512*r, 512*r + 512)
            xq = np.ascontiguousarray(
                x[b][qtok, :].T.reshape(32, 128, 512).transpose(1, 0, 2)).astype(bf)
            xkv = np.ascontiguousarray(
                x[b][kvtok, :].T.reshape(32, 128, 512).transpose(1, 0, 2)).astype(bf)
            m = dict(xq=xq, xkv=xkv, wq=wq, bq=bq, wk=wk, bk=bk, wv=wv,
                     bV=bV, w0=w0, bias_plane=plane, mask_diag=mask_diag)
            if not causal:
                m["maskT"] = maskT_perm
            in_maps.append(m)
            meta.append((b, r))
    return causal, in_maps, meta


def kernel(**inputs):
    causal, in_maps, meta = _prep(inputs)
    if causal not in _CACHE:
        _CACHE[causal] = build(causal)
    nc = _CACHE[causal]
    res = run_bass_kernel_spmd(nc, in_maps, core_ids=list(range(8)))
    out = np.empty((B, S, H), np.float32)
    for i, (b, r) in enumerate(meta):
        outT = res.results[i]["outT"]
        for hloc in range(8):
            hq = 2*r + 8*(hloc // 2) + (hloc % 2)
            out[b, 64*hq:64*hq+64, :] = outT[:, 64*hloc:64*hloc+64].T
    return out



# revision 6
# speedup vs baseline: 1.2420x; 1.2420x over previous
"""Trainium2 Bass kernel for nn_Attention_54520314855575.

GQA attention with raw row-major reshapes (faithful to reference). The raw
reshapes scramble heads/tokens such that each query head's 64 output rows are
disjoint across heads -> shard 8 ways (2 batches x 4 head-groups) with zero
collectives. Per core: 8 query heads (hq%8 in {2r, 2r+1}), K/V heads {2r,2r+1}.

v2 layout notes:
- q columns are sm-major within each 512-block (col l = 16*sm + u'), so all
  projection evictions write contiguous runs and the causal diagonal uses 4
  precomputed multiplicative 0/1 tiles applied to E after exp (bf16 DVE).
- t within a 128-tile: partition pnew = 16*cc + m holds t-offset 8*m + cc.
- V is projected naturally (t on partitions) and shuffled into Vsh with
  SBUF->SBUF stripe DMAs (no DRAM round trip); bias fused into the PSUM evict.
- softmax denominator: one tensor_reduce over E tiles + ones-matmul partition
  reduction + reciprocal_approx_fast + gpsimd broadcast.
- phase order V -> K -> Q so the V shuffle overlaps K/Q projections; out-proj
  weight/rhs pools live at outer scope so their DMAs prefetch across phases.
"""
import sys, os

for _p in ("/opt/trn_rl_repo",):
    if _p not in sys.path:
        sys.path.append(_p)

import numpy as np
import ml_dtypes

import concourse.bass as bass
import concourse.tile as tile
from concourse import bacc, mybir
from concourse.bass_utils import run_bass_kernel_spmd

BF16 = mybir.dt.bfloat16
F32 = mybir.dt.float32
F32R = mybir.dt.float32r

H = 4096; HQ = 32; HK = 8; HV = 8; DQ = 128; DV = 512; S = 2048; B = 2

_CACHE = {}


def build(causal: bool):
    nc = bacc.Bacc(None, target_bir_lowering=False, debug=False)

    xq_d = nc.declare_dram_parameter("xq", [128, 32, 512], BF16, isOutput=False)
    xkv_d = nc.declare_dram_parameter("xkv", [128, 32, 512], BF16, isOutput=False)
    wq_d = nc.declare_dram_parameter("wq", [8, 32, 128, 512], BF16, isOutput=False)
    bq_d = nc.declare_dram_parameter("bq", [128, 32], F32, isOutput=False)
    wk_d = nc.declare_dram_parameter("wk", [2, 32, 128, 512], BF16, isOutput=False)
    bk_d = nc.declare_dram_parameter("bk", [128, 8], F32, isOutput=False)
    wv_d = nc.declare_dram_parameter("wv", [8, 32, 128, 512], BF16, isOutput=False)
    bVn_d = nc.declare_dram_parameter("bVn", [128, 8, 512], BF16, isOutput=False)
    w0_d = nc.declare_dram_parameter("w0", [4, 128, 128, 1024], BF16, isOutput=False)
    b0_d = nc.declare_dram_parameter("b0", [128, 32], F32, isOutput=False)
    diagB_d = nc.declare_dram_parameter("diagB", [128, 4, 512], BF16, isOutput=False)
    if not causal:
        maskB_d = nc.declare_dram_parameter("maskB", [16, 128, 4, 512], BF16, isOutput=False)
    outT_d = nc.declare_dram_parameter("outT", [4096, 512], F32, isOutput=True)

    with tile.TileContext(nc) as tc:
        with tc.tile_pool(name="const", bufs=1) as constp, \
             tc.tile_pool(name="qkv", bufs=1) as qkvp, \
             tc.tile_pool(name="w0str", bufs=8) as w0p, \
             tc.tile_pool(name="rhsp", bufs=8) as rhp, \
             tc.tile_pool(name="dram", bufs=1, space="DRAM") as dramp:

            diagB_sb = constp.tile([128, 4, 512], BF16)
            nc.sync.dma_start(diagB_sb[:], diagB_d[:])
            bq_sb = constp.tile([128, 32], F32)
            nc.sync.dma_start(bq_sb[:], bq_d[:])
            bk_sb = constp.tile([128, 8], F32)
            nc.sync.dma_start(bk_sb[:], bk_d[:])
            b0_sb = constp.tile([128, 32], F32)
            nc.sync.dma_start(b0_sb[:], b0_d[:])
            ones_f = constp.tile([128, 1], F32)
            nc.vector.memset(ones_f[:], 1.0)
            ones_r = constp.tile([128, 1], F32R)
            nc.vector.tensor_copy(ones_r[:], ones_f[:])

            QT = qkvp.tile([128, 8, 4, 32, 16], BF16)   # [d, hd, c, sm, u']
            KT = qkvp.tile([128, 2, 16, 8, 16], BF16)   # [d, hd, i4, cc, m]
            Vsh = qkvp.tile([128, 2, 16, 512], BF16)    # [pnew, hd, i4, d]
            ctx_dram = dramp.tile([32, 4, 128, 512], BF16)  # [sm, dd, dpart, s']

            # ---------------- Phase 1: projections (V -> K -> Q) -----------
            with tc.tile_pool(name="xres", bufs=1) as xp, \
                 tc.tile_pool(name="wstr", bufs=6) as wp, \
                 tc.tile_pool(name="vtmp", bufs=4) as vtp, \
                 tc.tile_pool(name="pps", bufs=8, space="PSUM") as pps:

                xkv_sb = xp.tile([128, 32, 512], BF16)
                for g_ in range(4):
                    nc.sync.dma_start(xkv_sb[:, 8*g_:8*g_+8, :], xkv_d[:, 8*g_:8*g_+8, :])
                xq_sb = xp.tile([128, 32, 512], BF16)
                nc.sync.dma_start(xq_sb[:], xq_d[:])
                bVn_sb = xp.tile([128, 8, 512], BF16)
                nc.sync.dma_start(bVn_sb[:], bVn_d[:])

                # V natural: lhsT = xkv block (h128, tok128), rhs = wv (h128, c512)
                # acc[tt][p, d] = V[t = 1024*(tt%2) + 8p + ccg, d], head tt//2
                for ccg in range(8):
                    acc = [pps.tile([128, 512], F32, tag="pj", name=f"pj{_}") for _ in range(4)]
                    for h in range(32):
                        wsl = wp.tile([128, 512], BF16, tag="w")
                        nc.sync.dma_start(wsl[:], wv_d[ccg, h, :, :])
                        for tt in range(4):
                            nc.tensor.matmul(acc[tt][:], xkv_sb[:, h, 128*tt:128*tt+128],
                                             wsl[:], start=(h == 0), stop=(h == 31))
                    for tt in range(4):
                        vnat = vtp.tile([128, 512], BF16, tag="vn")
                        nc.vector.tensor_add(vnat[:], acc[tt][:], bVn_sb[:, ccg, :])
                        j0, half = tt // 2, tt % 2
                        for i4p in range(8):
                            nc.sync.dma_start(
                                Vsh[16*ccg:16*ccg+16, j0, 8*half+i4p, :],
                                vnat[16*i4p:16*i4p+16, :])

                # KT: acc[i][p=d, tok=256*hd+16*i4+m] -> KT[d, hd, ct, i4, m]
                for cg in range(2):
                    acc = [pps.tile([128, 512], F32, tag="pj", name=f"pj{_}") for _ in range(4)]
                    for h in range(32):
                        wsl = wp.tile([128, 512], BF16, tag="w")
                        nc.sync.dma_start(wsl[:], wk_d[cg, h, :, :])
                        for i in range(4):
                            nc.tensor.matmul(acc[i][:], wsl[:, 128*i:128*i+128],
                                             xkv_sb[:, h, :], start=(h == 0), stop=(h == 31))
                    for i in range(4):
                        ct = 4*cg + i  # == cc
                        nc.vector.tensor_scalar_add(
                            KT[:, :, :, ct, :],
                            acc[i][:].rearrange("p (hd i4 m) -> p hd i4 m", hd=2, i4=16),
                            bk_sb[:, ct:ct+1])

                # QT: acc[i][p=d, tok=64*hd+u] -> QT[d, ct, hd, u]
                for cg in range(8):
                    acc = [pps.tile([128, 512], F32, tag="pj", name=f"pj{_}") for _ in range(4)]
                    for h in range(32):
                        wsl = wp.tile([128, 512], BF16, tag="w")
                        nc.sync.dma_start(wsl[:], wq_d[cg, h, :, :])
                        for i in range(4):
                            nc.tensor.matmul(acc[i][:], wsl[:, 128*i:128*i+128],
                                             xq_sb[:, h, :], start=(h == 0), stop=(h == 31))
                    for i in range(4):
                        ct = 4*cg + i  # == sm
                        nc.vector.tensor_scalar_add(
                            QT[:, :, :, ct, :],
                            acc[i][:].rearrange("p (hd c u) -> p hd c u", hd=8, c=4),
                            bq_sb[:, ct:ct+1])

            # ---------------- Phase 2: attention ----------------
            with tc.tile_pool(name="esb", bufs=2) as ep, \
                 tc.tile_pool(name="nrm", bufs=2) as np_, \
                 tc.tile_pool(name="cev", bufs=4) as cevp, \
                 tc.tile_pool(name="mstr", bufs=4) as mp, \
                 tc.tile_pool(name="aps", bufs=1, space="PSUM") as aps:

                for hloc in range(8):
                    k, j0 = hloc // 2, hloc % 2
                    for c in range(4):
                        nt = 4*c + 4 if causal else 16
                        E = ep.tile([128, 16, 512], BF16, tag="E")
                        rs_ps = aps.tile([1, 512], F32, tag="rs")
                        pctx = [aps.tile([128, 512], F32, tag=f"ctx{dd}", name=f"ctx{dd}")
                                for dd in range(4)]
                        for i4 in range(nt):
                            sc_ps = aps.tile([128, 512], F32, tag="sc", bufs=2)
                            nc.tensor.matmul(
                                sc_ps[:], KT[:, j0, i4, :, :].opt(),
                                QT[:, hloc, c, :, :].opt(), start=True, stop=True)
                            nc.scalar.activation(E[:, i4, :], sc_ps[:],
                                                 mybir.ActivationFunctionType.Exp)
                            if causal:
                                if 4*c <= i4:
                                    nc.vector.tensor_mul(E[:, i4, :], E[:, i4, :],
                                                         diagB_sb[:, i4 - 4*c, :])
                            else:
                                msk = mp.tile([128, 512], BF16, tag="mk")
                                nc.sync.dma_start(msk[:], maskB_d[i4, :, c, :])
                                nc.vector.tensor_mul(E[:, i4, :], E[:, i4, :], msk[:])
                            for dd in range(4):
                                nc.tensor.matmul(pctx[dd][:],
                                                 Vsh[:, j0, i4, 128*dd:128*dd+128],
                                                 E[:, i4, :],
                                                 start=(i4 == 0), stop=(i4 == nt-1))
                        Esum = np_.tile([128, 512], F32R, tag="es")
                        with nc.allow_low_precision(reason="f32r is 32-bit"):
                            nc.vector.tensor_reduce(
                                Esum[:], E[:, 0:nt, :].rearrange("p i q -> p q i"),
                                axis=mybir.AxisListType.X, op=mybir.AluOpType.add)
                        nc.tensor.matmul(rs_ps[:], ones_r[:], Esum[:],
                                         start=True, stop=True)
                        rc1_sb = np_.tile([1, 512], F32, tag="rc1")
                        nc.vector.reciprocal_approx_fast(rc1_sb[:], rs_ps[:])
                        rc_sb = np_.tile([128, 512], F32, tag="rc")
                        nc.gpsimd.partition_broadcast(rc_sb[:], rc1_sb[:])
                        s0 = 64*hloc + 16*c
                        for dd in range(4):
                            cev = cevp.tile([128, 512], BF16, tag="cev")
                            nc.vector.tensor_mul(cev[:], pctx[dd][:], rc_sb[:])
                            src = cev[:].rearrange("p (sm u) -> p sm u", u=16)
                            dst = ctx_dram[:, dd, :, s0:s0+16].rearrange("sm p u -> p sm u")
                            nc.sync.dma_start(dst.opt(), src.opt())

            # ---------------- Phase 3: output projection ----------------
            with tc.tile_pool(name="evo", bufs=4) as evp, \
                 tc.tile_pool(name="wps", bufs=1, space="PSUM") as wps:

                for og in range(4):
                    pout = [wps.tile([128, 512], F32, tag=f"o{o}", name=f"po{o}")
                            for o in range(8)]
                    for ft in range(128):
                        sm, dd = ft // 4, ft % 4
                        rhs = rhp.tile([128, 512], BF16, tag="rhs")
                        nc.sync.dma_start(rhs[:], ctx_dram[sm, dd, :, :])
                        wsl = w0p.tile([128, 1024], BF16, tag="w0")
                        nc.sync.dma_start(wsl[:], w0_d[og, ft, :, :])
                        for o in range(8):
                            nc.tensor.matmul(pout[o][:], wsl[:, 128*o:128*o+128], rhs[:],
                                             start=(ft == 0), stop=(ft == 127))
                    for o in range(8):
                        orow = 1024*og + 128*o
                        res = evp.tile([128, 512], F32, tag="res")
                        nc.vector.tensor_scalar_add(res[:], pout[o][:],
                                                    b0_sb[:, 8*og+o:8*og+o+1])
                        nc.sync.dma_start(outT_d[orow:orow+128, :], res[:])

    nc.compile()
    return nc


def _tile_w(wT, ncg):
    """(4096h, ncg*512c) -> (ncg, 32, 128, 512) contiguous slabs."""
    hdim = wT.shape[0]
    return np.ascontiguousarray(
        wT.reshape(hdim // 128, 128, ncg, 512).transpose(2, 0, 1, 3))


def _prep(inputs):
    x = np.asarray(inputs["x"], np.float32)
    mask = np.asarray(inputs["mask"]).astype(bool)
    WQ_w = np.asarray(inputs["WQ_w"], np.float32); WQ_b = np.asarray(inputs["WQ_b"], np.float32)
    WK_w = np.asarray(inputs["WK_w"], np.float32); WK_b = np.asarray(inputs["WK_b"], np.float32)
    WV_w = np.asarray(inputs["WV_w"], np.float32); WV_b = np.asarray(inputs["WV_b"], np.float32)
    W0_w = np.asarray(inputs["W0_w"], np.float32); W0_b = np.asarray(inputs["W0_b"], np.float32)

    causal = bool(np.array_equal(mask, np.triu(np.ones((S, S), bool), k=1)))

    bf = ml_dtypes.bfloat16
    sc = 1.0 / np.sqrt(DQ)
    wq = _tile_w(np.ascontiguousarray((WQ_w * sc).T).astype(bf), 8)
    wk = _tile_w(np.ascontiguousarray(WK_w.T).astype(bf), 2)
    wv = _tile_w(np.ascontiguousarray(WV_w.T).astype(bf), 8)
    w0T = np.ascontiguousarray(W0_w.T).astype(bf)           # (16384, 4096)
    w0 = np.ascontiguousarray(
        w0T.reshape(128, 128, 4, 1024).transpose(2, 0, 1, 3))  # (og, ft, p, 1024)

    # t-permutation within a 128-tile: partition p holds t-offset 8*(p%16) + p//16
    pnew = np.arange(128)
    t_of_p = 8*(pnew % 16) + pnew // 16                     # (128,)
    # q-permutation within a 512-block: col l holds q-offset 32*(l%16) + l//16
    l_ = np.arange(512)
    q_of_l = 32*(l_ % 16) + l_ // 16                        # (512,)

    # V bias by channel group: bVn[p, cc, d] = WV_b[512*cc + d]
    bVn = np.broadcast_to(WV_b.reshape(1, 8, 512), (128, 8, 512)).astype(bf)
    bVn = np.ascontiguousarray(bVn)

    # multiplicative diagonal mask tiles: keep iff 128*jd + t_of_p <= q_of_l
    jd_ = np.arange(4)
    diagB = (128*jd_[None, :, None] + t_of_p[:, None, None]
             <= q_of_l[None, None, :]).astype(bf)           # (128, 4, 512)
    diagB = np.ascontiguousarray(diagB)

    maskB_perm = None
    if not causal:
        # maskB[i4, p, c, l] = 0/1 keep-multiplier, t = 128*i4 + t_of_p[p],
        # q = 512*c + q_of_l[l]; mask[q, t] True = masked
        keep = (~mask.T).astype(np.float32)                 # (t, q)
        m4 = keep.reshape(16, 128, 4, 512)
        maskB_perm = np.ascontiguousarray(
            m4[:, t_of_p, :, :][:, :, :, q_of_l]).astype(bf)

    def fold(v, ntile):
        return np.ascontiguousarray(v.reshape(ntile, 128).T).astype(np.float32)

    bq = fold(WQ_b * sc, 32)
    bk = fold(WK_b, 8)
    b0 = fold(W0_b, 32)

    in_maps = []
    meta = []
    for b in range(B):
        for r in range(4):
            qtok = np.concatenate(
                [np.arange(512*kk + 128*r, 512*kk + 128*r + 128) for kk in range(4)])
            kvtok = np.arange(512*r, 512*r + 512)
            xq = np.ascontiguousarray(
                x[b][qtok, :].T.reshape(32, 128, 512).transpose(1, 0, 2)).astype(bf)
            xkv = np.ascontiguousarray(
                x[b][kvtok, :].T.reshape(32, 128, 512).transpose(1, 0, 2)).astype(bf)
            m = dict(xq=xq, xkv=xkv, wq=wq, bq=bq, wk=wk, bk=bk, wv=wv,
                     bVn=bVn, w0=w0, b0=b0, diagB=diagB)
            if not causal:
                m["maskB"] = maskB_perm
            in_maps.append(m)
            meta.append((b, r))
    return causal, in_maps, meta


def kernel(**inputs):
    causal, in_maps, meta = _prep(inputs)
    if causal not in _CACHE:
        _CACHE[causal] = build(causal)
    nc = _CACHE[causal]
    res = run_bass_kernel_spmd(nc, in_maps, core_ids=list(range(8)))
    out = np.empty((B, S, H), np.float32)
    for i, (b, r) in enumerate(meta):
        outT = res.results[i]["outT"]
        for hloc in range(8):
            hq = 2*r + 8*(hloc // 2) + (hloc % 2)
            out[b, 64*hq:64*hq+64, :] = outT[:, 64*hloc:64*hloc+64].T
    return out


# revision 11
# speedup vs baseline: 1.3217x; 1.0642x over previous
"""Trainium2 Bass kernel for nn_Attention_54520314855575.

GQA attention with raw row-major reshapes (faithful to reference). The raw
reshapes scramble heads/tokens such that each query head's 64 output rows are
disjoint across heads -> shard 8 ways (2 batches x 4 head-groups) with zero
collectives. Per core: 8 query heads (hq%8 in {2r, 2r+1}), K/V heads {2r,2r+1}.

v2 layout notes:
- q columns are sm-major within each 512-block (col l = 16*sm + u'), so all
  projection evictions write contiguous runs and the causal diagonal uses 4
  precomputed multiplicative 0/1 tiles applied to E after exp (bf16 DVE).
- t within a 128-tile: partition pnew = 16*cc + m holds t-offset 8*m + cc.
- V is projected naturally (t on partitions) and shuffled into Vsh with
  SBUF->SBUF stripe DMAs (no DRAM round trip); bias fused into the PSUM evict.
- softmax denominator: one tensor_reduce over E tiles + ones-matmul partition
  reduction + reciprocal_approx_fast + gpsimd broadcast.
- phase order V -> K -> Q so the V shuffle overlaps K/Q projections; out-proj
  weight/rhs pools live at outer scope so their DMAs prefetch across phases.
"""
import sys, os

for _p in ("/opt/trn_rl_repo",):
    if _p not in sys.path:
        sys.path.append(_p)

import numpy as np
import ml_dtypes

import concourse.bass as bass
import concourse.tile as tile
from concourse import bacc, mybir
from concourse.bass_utils import run_bass_kernel_spmd

BF16 = mybir.dt.bfloat16
F32 = mybir.dt.float32
F32R = mybir.dt.float32r

H = 4096; HQ = 32; HK = 8; HV = 8; DQ = 128; DV = 512; S = 2048; B = 2

_CACHE = {}


def build(causal: bool):
    nc = bacc.Bacc(None, target_bir_lowering=False, debug=False)

    xq_d = nc.declare_dram_parameter("xq", [128, 32, 512], BF16, isOutput=False)
    xkv_d = nc.declare_dram_parameter("xkv", [128, 32, 512], BF16, isOutput=False)
    wq_d = nc.declare_dram_parameter("wq", [8, 32, 128, 512], BF16, isOutput=False)
    bq_d = nc.declare_dram_parameter("bq", [128, 32], F32, isOutput=False)
    wk_d = nc.declare_dram_parameter("wk", [2, 32, 128, 512], BF16, isOutput=False)
    bk_d = nc.declare_dram_parameter("bk", [128, 8], F32, isOutput=False)
    wv_d = nc.declare_dram_parameter("wv", [8, 32, 128, 512], BF16, isOutput=False)
    bVn_d = nc.declare_dram_parameter("bVn", [128, 8, 512], BF16, isOutput=False)
    w0_d = nc.declare_dram_parameter("w0", [4, 128, 128, 1024], BF16, isOutput=False)
    b0_d = nc.declare_dram_parameter("b0", [128, 32], F32, isOutput=False)
    diagB_d = nc.declare_dram_parameter("diagB", [128, 4, 512], BF16, isOutput=False)
    if not causal:
        maskB_d = nc.declare_dram_parameter("maskB", [16, 128, 4, 512], BF16, isOutput=False)
    outT_d = nc.declare_dram_parameter("outT", [4096, 512], F32, isOutput=True)

    with tile.TileContext(nc) as tc:
        with tc.tile_pool(name="const", bufs=1) as constp, \
             tc.tile_pool(name="qkv", bufs=1) as qkvp, \
             tc.tile_pool(name="w0str", bufs=8) as w0p, \
             tc.tile_pool(name="rhsp", bufs=8) as rhp, \
             tc.tile_pool(name="dram", bufs=1, space="DRAM") as dramp:

            diagB_sb = constp.tile([128, 4, 512], BF16)
            nc.sync.dma_start(diagB_sb[:], diagB_d[:])
            bq_sb = constp.tile([128, 32], F32)
            nc.sync.dma_start(bq_sb[:], bq_d[:])
            bk_sb = constp.tile([128, 8], F32)
            nc.sync.dma_start(bk_sb[:], bk_d[:])
            b0_sb = constp.tile([128, 32], F32)
            nc.sync.dma_start(b0_sb[:], b0_d[:])
            ones_bf = constp.tile([128, 1], BF16)
            nc.vector.memset(ones_bf[:], 1.0)

            QT = qkvp.tile([128, 8, 4, 32, 16], BF16)   # [d, hd, c, sm, u']
            KT = qkvp.tile([128, 2, 16, 8, 16], BF16)   # [d, hd, i4, cc, m]
            Vsh = qkvp.tile([128, 2, 16, 512], BF16)    # [pnew, hd, i4, d]
            ctx_dram = dramp.tile([32, 4, 128, 512], BF16)  # [sm, dd, dpart, s']

            # ---------------- Phase 1: projections (V -> K -> Q) -----------
            with tc.tile_pool(name="xres", bufs=1) as xp, \
                 tc.tile_pool(name="wstr", bufs=6) as wp, \
                 tc.tile_pool(name="vtmp", bufs=4) as vtp, \
                 tc.tile_pool(name="pps", bufs=8, space="PSUM") as pps:

                xkv_sb = xp.tile([128, 32, 512], BF16)
                for g_ in range(4):
                    nc.sync.dma_start(xkv_sb[:, 8*g_:8*g_+8, :], xkv_d[:, 8*g_:8*g_+8, :])
                bVn_sb = xp.tile([128, 8, 512], BF16)
                nc.sync.dma_start(bVn_sb[:], bVn_d[:])

                # V natural: lhsT = xkv block (h128, tok128), rhs = wv (h128, c512)
                # acc[tt][p, d] = V[t = 1024*(tt%2) + 8p + ccg, d], head tt//2
                for ccg in range(8):
                    acc = [pps.tile([128, 512], F32, tag="pj", name=f"pj{_}") for _ in range(4)]
                    for h in range(32):
                        wsl = wp.tile([128, 512], BF16, tag="w")
                        nc.sync.dma_start(wsl[:], wv_d[ccg, h, :, :])
                        for tt in range(4):
                            nc.tensor.matmul(acc[tt][:], xkv_sb[:, h, 128*tt:128*tt+128],
                                             wsl[:], start=(h == 0), stop=(h == 31))
                    for tt in range(4):
                        vnat = vtp.tile([128, 512], BF16, tag="vn")
                        nc.vector.tensor_add(vnat[:], acc[tt][:], bVn_sb[:, ccg, :])
                        j0, half = tt // 2, tt % 2
                        for i4p in range(8):
                            nc.gpsimd.dma_start(
                                Vsh[16*ccg:16*ccg+16, j0, 8*half+i4p, :],
                                vnat[16*i4p:16*i4p+16, :])

                xq_sb = xp.tile([128, 32, 512], BF16)
                for g_ in range(4):
                    nc.sync.dma_start(xq_sb[:, 8*g_:8*g_+8, :], xq_d[:, 8*g_:8*g_+8, :])

                # KT: acc[i][p=d, tok=256*hd+16*i4+m] -> KT[d, hd, ct, i4, m]
                for cg in range(2):
                    acc = [pps.tile([128, 512], F32, tag="pj", name=f"pj{_}") for _ in range(4)]
                    for h in range(32):
                        wsl = wp.tile([128, 512], BF16, tag="w")
                        nc.sync.dma_start(wsl[:], wk_d[cg, h, :, :])
                        for i in range(4):
                            nc.tensor.matmul(acc[i][:], wsl[:, 128*i:128*i+128],
                                             xkv_sb[:, h, :], start=(h == 0), stop=(h == 31))
                    for i in range(4):
                        ct = 4*cg + i  # == cc
                        nc.vector.tensor_scalar_add(
                            KT[:, :, :, ct, :],
                            acc[i][:].rearrange("p (hd i4 m) -> p hd i4 m", hd=2, i4=16),
                            bk_sb[:, ct:ct+1])

                # QT: acc[i][p=d, tok=64*hd+u] -> QT[d, ct, hd, u]
                for cg in range(8):
                    acc = [pps.tile([128, 512], F32, tag="pj", name=f"pj{_}") for _ in range(4)]
                    for h in range(32):
                        wsl = wp.tile([128, 512], BF16, tag="w")
                        nc.sync.dma_start(wsl[:], wq_d[cg, h, :, :])
                        for i in range(4):
                            nc.tensor.matmul(acc[i][:], wsl[:, 128*i:128*i+128],
                                             xq_sb[:, h, :], start=(h == 0), stop=(h == 31))
                    for i in range(4):
                        ct = 4*cg + i  # == sm
                        nc.vector.tensor_scalar_add(
                            QT[:, :, :, ct, :],
                            acc[i][:].rearrange("p (hd c u) -> p hd c u", hd=8, c=4),
                            bq_sb[:, ct:ct+1])

            # ---------------- Phase 2: attention ----------------
            with tc.tile_pool(name="esb", bufs=2) as ep, \
                 tc.tile_pool(name="nrm", bufs=2) as np_, \
                 tc.tile_pool(name="cev", bufs=4) as cevp, \
                 tc.tile_pool(name="mstr", bufs=4) as mp, \
                 tc.tile_pool(name="aps", bufs=1, space="PSUM") as aps:

                for hloc in range(8):
                    k, j0 = hloc // 2, hloc % 2
                    for c in range(4):
                        nt = 4*c + 4 if causal else 16
                        E = ep.tile([128, 16, 512], BF16, tag="E")
                        rs_ps = aps.tile([1, 512], F32, tag="rs", bufs=2)
                        pctx = [aps.tile([128, 512], F32, tag=f"ctx{dd}", name=f"ctx{dd}")
                                for dd in range(4)]
                        for i4 in range(nt):
                            sc_ps = aps.tile([128, 512], F32, tag="sc", bufs=2)
                            nc.tensor.matmul(
                                sc_ps[:], KT[:, j0, i4, :, :].opt(),
                                QT[:, hloc, c, :, :].opt(), start=True, stop=True)
                            nc.scalar.activation(E[:, i4, :], sc_ps[:],
                                                 mybir.ActivationFunctionType.Exp)
                            if causal:
                                if 4*c <= i4:
                                    nc.vector.tensor_mul(E[:, i4, :], E[:, i4, :],
                                                         diagB_sb[:, i4 - 4*c, :])
                            else:
                                msk = mp.tile([128, 512], BF16, tag="mk")
                                nc.sync.dma_start(msk[:], maskB_d[i4, :, c, :])
                                nc.vector.tensor_mul(E[:, i4, :], E[:, i4, :], msk[:])
                            nc.tensor.matmul(rs_ps[:], ones_bf[:], E[:, i4, :],
                                             start=(i4 == 0), stop=(i4 == nt-1))
                            for dd in range(4):
                                nc.tensor.matmul(pctx[dd][:],
                                                 Vsh[:, j0, i4, 128*dd:128*dd+128],
                                                 E[:, i4, :],
                                                 start=(i4 == 0), stop=(i4 == nt-1))
                        rc1_sb = np_.tile([1, 512], F32, tag="rc1")
                        nc.vector.reciprocal_approx_fast(rc1_sb[:], rs_ps[:])
                        rc_sb = np_.tile([128, 512], F32, tag="rc")
                        nc.gpsimd.partition_broadcast(rc_sb[:], rc1_sb[:])
                        s0 = 64*hloc + 16*c
                        for dd in range(4):
                            cev = cevp.tile([128, 512], BF16, tag="cev")
                            nc.vector.tensor_mul(cev[:], pctx[dd][:], rc_sb[:])
                            src = cev[:].rearrange("p (sm u) -> p sm u", u=16)
                            dst = ctx_dram[:, dd, :, s0:s0+16].rearrange("sm p u -> p sm u")
                            nc.sync.dma_start(dst.opt(), src.opt())

            # ---------------- Phase 3: output projection ----------------
            with tc.tile_pool(name="evo", bufs=4) as evp, \
                 tc.tile_pool(name="wps", bufs=1, space="PSUM") as wps:

                for og in range(4):
                    pout = [wps.tile([128, 512], F32, tag=f"o{o}", name=f"po{o}")
                            for o in range(8)]
                    for ft in range(128):
                        sm, dd = ft // 4, ft % 4
                        rhs = rhp.tile([128, 512], BF16, tag="rhs")
                        nc.sync.dma_start(rhs[:], ctx_dram[sm, dd, :, :])
                        wsl = w0p.tile([128, 1024], BF16, tag="w0")
                        nc.sync.dma_start(wsl[:], w0_d[og, ft, :, :])
                        for o in range(8):
                            nc.tensor.matmul(pout[o][:], wsl[:, 128*o:128*o+128], rhs[:],
                                             start=(ft == 0), stop=(ft == 127))
                    for o in range(8):
                        orow = 1024*og + 128*o
                        res = evp.tile([128, 512], F32, tag="res")
                        nc.vector.tensor_scalar_add(res[:], pout[o][:],
                                                    b0_sb[:, 8*og+o:8*og+o+1])
                        nc.sync.dma_start(outT_d[orow:orow+128, :], res[:])

    nc.compile()
    return nc


def _tile_w(wT, ncg):
    """(4096h, ncg*512c) -> (ncg, 32, 128, 512) contiguous slabs."""
    hdim = wT.shape[0]
    return np.ascontiguousarray(
        wT.reshape(hdim // 128, 128, ncg, 512).transpose(2, 0, 1, 3))


def _prep(inputs):
    x = np.asarray(inputs["x"], np.float32)
    mask = np.asarray(inputs["mask"]).astype(bool)
    WQ_w = np.asarray(inputs["WQ_w"], np.float32); WQ_b = np.asarray(inputs["WQ_b"], np.float32)
    WK_w = np.asarray(inputs["WK_w"], np.float32); WK_b = np.asarray(inputs["WK_b"], np.float32)
    WV_w = np.asarray(inputs["WV_w"], np.float32); WV_b = np.asarray(inputs["WV_b"], np.float32)
    W0_w = np.asarray(inputs["W0_w"], np.float32); W0_b = np.asarray(inputs["W0_b"], np.float32)

    causal = bool(np.array_equal(mask, np.triu(np.ones((S, S), bool), k=1)))

    bf = ml_dtypes.bfloat16
    sc = 1.0 / np.sqrt(DQ)
    wq = _tile_w(np.ascontiguousarray((WQ_w * sc).T).astype(bf), 8)
    wk = _tile_w(np.ascontiguousarray(WK_w.T).astype(bf), 2)
    wv = _tile_w(np.ascontiguousarray(WV_w.T).astype(bf), 8)
    w0T = np.ascontiguousarray(W0_w.T).astype(bf)           # (16384, 4096)
    w0 = np.ascontiguousarray(
        w0T.reshape(128, 128, 4, 1024).transpose(2, 0, 1, 3))  # (og, ft, p, 1024)

    # t-permutation within a 128-tile: partition p holds t-offset 8*(p%16) + p//16
    pnew = np.arange(128)
    t_of_p = 8*(pnew % 16) + pnew // 16                     # (128,)
    # q-permutation within a 512-block: col l holds q-offset 32*(l%16) + l//16
    l_ = np.arange(512)
    q_of_l = 32*(l_ % 16) + l_ // 16                        # (512,)

    # V bias by channel group: bVn[p, cc, d] = WV_b[512*cc + d]
    bVn = np.broadcast_to(WV_b.reshape(1, 8, 512), (128, 8, 512)).astype(bf)
    bVn = np.ascontiguousarray(bVn)

    # multiplicative diagonal mask tiles: keep iff 128*jd + t_of_p <= q_of_l
    jd_ = np.arange(4)
    diagB = (128*jd_[None, :, None] + t_of_p[:, None, None]
             <= q_of_l[None, None, :]).astype(bf)           # (128, 4, 512)
    diagB = np.ascontiguousarray(diagB)

    maskB_perm = None
    if not causal:
        # maskB[i4, p, c, l] = 0/1 keep-multiplier, t = 128*i4 + t_of_p[p],
        # q = 512*c + q_of_l[l]; mask[q, t] True = masked
        keep = (~mask.T).astype(np.float32)                 # (t, q)
        m4 = keep.reshape(16, 128, 4, 512)
        maskB_perm = np.ascontiguousarray(
            m4[:, t_of_p, :, :][:, :, :, q_of_l]).astype(bf)

    def fold(v, ntile):
        return np.ascontiguousarray(v.reshape(ntile, 128).T).astype(np.float32)

    bq = fold(WQ_b * sc, 32)
    bk = fold(WK_b, 8)
    b0 = fold(W0_b, 32)

    in_maps = []
    meta = []
    for b in range(B):
        for r in range(4):
            qtok = np.concatenate(
                [np.arange(512*kk + 128*r, 512*kk + 128*r + 128) for kk in range(4)])
            kvtok = np.arange(512*r, 512*r + 512)
            xq = np.ascontiguousarray(
                x[b][qtok, :].T.reshape(32, 128, 512).transpose(1, 0, 2)).astype(bf)
            xkv = np.ascontiguousarray(
                x[b][kvtok, :].T.reshape(32, 128, 512).transpose(1, 0, 2)).astype(bf)
            m = dict(xq=xq, xkv=xkv, wq=wq, bq=bq, wk=wk, bk=bk, wv=wv,
                     bVn=bVn, w0=w0, b0=b0, diagB=diagB)
            if not causal:
                m["maskB"] = maskB_perm
            in_maps.append(m)
            meta.append((b, r))
    return causal, in_maps, meta


def kernel(**inputs):
    causal, in_maps, meta = _prep(inputs)
    if causal not in _CACHE:
        _CACHE[causal] = build(causal)
    nc = _CACHE[causal]
    res = run_bass_kernel_spmd(nc, in_maps, core_ids=list(range(8)))
    out = np.empty((B, S, H), np.float32)
    for i, (b, r) in enumerate(meta):
        outT = res.results[i]["outT"]
        for hloc in range(8):
            hq = 2*r + 8*(hloc // 2) + (hloc % 2)
            out[b, 64*hq:64*hq+64, :] = outT[:, 64*hloc:64*hloc+64].T
    return out
